# revision 42
# baseline (speedup 1.0000x reference)
"""Binarized 3x3 conv (GeneralConv2d) on 8 NeuronCores — partial-residual fp8.

y[b,o,h,w] = mean_abs(w[o]) * sum_{c,kh,kw} sign(w[o,c,kh,kw]) * x[b,c,h+kh-1,w+kw-1]

Data-parallel over batch: 4 images per core on 8 cores; the tiny binarized
weight is replicated.  The conv runs on the tensor engine as fp8e4 DoubleRow
matmuls (256-deep contraction per instruction: the two in-channel chunks
ride in the k-tile pair dim), one halo-free 448-column window (8 rows x 56,
strided rhs AP) per instruction.

x ships from the host as an e4m3 hi/lo pair (hi = e4m3(x), lo = e4m3(x-hi),
both exact round-to-nearest) so no on-device residual math is needed.  The
error budget (rel 2e-2) does not require the full lo pass: only LO_TAPS of
the nine 3x3 taps get the lo-residual correction, which cuts the matmul
stream from 18 to 9+len(LO_TAPS) instructions per output chunk.  Error on
the fixed seeded inputs, measured on HW (bit-identical to the numpy model):
LO_TAPS=(0,2,4,8) -> rel 1.68e-2; (0,2,4,6,8) -> 1.66e-2; all nine -> 7e-4.

The sign matrix is computed on device as (w>=0)-0.5 = +-0.5 in a single DVE
op per tap-third (fp8-exact); the missing factor of 2 folds into the
per-channel eviction scale (2/CKK instead of 1/CKK).

Schedule notes (all verified against TimelineSim span traces):
- Dummy matmuls on a zeroed scratch keep the tensor engine continuously
  busy from t~1.5us until the real stream starts, so the p-state ramp
  (low/mid clock at the start of a busy run) is burnt on junk and every
  real matmul runs at full clock.
- DMA wire time is a single shared ~360GB/s resource and every DMA has
  ~2.8us of fixed issue+semaphore latency, so the startup-critical pieces
  (oo0 sign sources, img0 head) are spread over the sync/scalar/Pool
  queues in wire-deadline order, and everything else (oo1 signs, scale
  sources, img1) rides the Pool/SWDGE stream behind them, deadline-ordered.
- img0's x buffers are two overlapping head/rest tiles (chunks 0-2 read
  the head tile, rows>=23 the rest tile) so the first chunks gate only on
  the head DMAs; imgs 2-3 are prefetched at the middle of the previous
  image's oo1 pass.
- The first two chunks run hi-pass A/B style (both hi passes before the
  first lo taps) to hide the lo-head DMA latency.
- Evictions (PSUM scale-mul) run exclusively on ACT; stores ride the sync
  HWDGE queue.  The very last chunk is split 6+2 with the tiny tail stored
  via the otherwise-idle Pool/SWDGE queue, shortening the end-of-kernel
  drain chain.

Host-side layout prep (data movement / dtype casts only): x channels are
halo-padded to flat [58*58+2] lines so every load DMA is one contiguous
piece per partition; the weight ships transposed ([ckk, out]) as
clip(w*2^24, +-224) cast to e4m3 — a sign-exact monotone map that keeps
every weight a finite fp8 normal at half the bf16 wire cost — plus a bf16
copy for the on-device mean-abs scale reduction.
"""

import numpy as np

from contextlib import ExitStack

import concourse.mybir as mybir
from concourse import bacc
import concourse.tile as tile

dt = mybir.dt
OUT_C = 256
IN_C = 256
KH = KW = 3
KK = KH * KW           # 9
CKK = IN_C * KK        # 2304
P = 128
CC = IN_C // P         # 2 in-channel chunks (the DoubleRow k-tile pair)
OO = OUT_C // P        # 2 out-channel chunks

# Taps (kh*3+kw) that get the lo-residual correction pass.
LO_TAPS = (0, 2, 4, 8)
STRIDED = True         # 448-col halo-free matmuls (4-dim rhs AP)
N_WARM = 30            # dummy matmuls covering the p-state ramp window


def _build_conv_nc(imgs: int, H: int, W: int, hchunk: int = 8, psum_bufs: int = 8,
                   lo_taps=LO_TAPS, strided=STRIDED, n_warm=N_WARM):
    assert H % hchunk == 0
    nch = H // hchunk
    Hp, Wp = H + 2, W + 2
    FLAT = Hp * Wp         # 3364
    FPAD = FLAT + 2        # +2 so the last tap window stays in-bounds
    NMM = KK + len(lo_taps)
    NT3 = KK // 3          # tap-thirds
    nc = bacc.Bacc("TRN2", target_bir_lowering=False, debug=False,
                   enable_asserts=False, num_devices=8)
    xh_d = nc.declare_dram_parameter("xh", [imgs, IN_C, FPAD], dt.float8e4,
                                     isOutput=False)
    xl_d = nc.declare_dram_parameter("xl", [imgs, IN_C, FPAD], dt.float8e4,
                                     isOutput=False)
    w_d = nc.declare_dram_parameter("w", [OUT_C, CKK], dt.bfloat16, isOutput=False)
    wt_d = nc.declare_dram_parameter("wt", [OO, CC, P, KK * P], dt.float8e4,
                                     isOutput=False)
    y = nc.declare_dram_parameter("y", [imgs, OUT_C, H, W], dt.float32, isOutput=True)

    # img0 head/rest split: head covers chunks 0..2 (windows end <= 1508),
    # rest covers rows >= 23 (chunks 3..6 read windows in [1392, 3364]).
    HEAD = 1510
    RST = 1334             # rest tile holds flat cols [RST:FPAD]

    with tile.TileContext(nc) as tc, ExitStack() as ctx:
        wprep = ctx.enter_context(tc.tile_pool(name="wprep", bufs=1))
        w_sb = [[wprep.tile([P, CKK // 2], dt.bfloat16, name=f"w_sb{o}_{h}")
                 for h in range(2)] for o in range(OO)]
        scale_p = [wprep.tile([P, 2], dt.float32, name=f"scale_p{o}")
                   for o in range(OO)]
        wts_sb = [[wprep.tile([P, KK, P], dt.float8e4, name=f"wts{o}_{c}")
                   for c in range(CC)] for o in range(OO)]
        wt8 = [[wprep.tile([P, CC, 3, P], dt.float8e4, name=f"wt8_{o}_{t}")
                for t in range(NT3)] for o in range(OO)]
        scale_sb = [wprep.tile([P, 1], dt.float32, name=f"scale{o}")
                    for o in range(OO)]
        scratch = wprep.tile([P, CC, 256], dt.float8e4, name="scratch")
        nc.vector.memset(scratch, 0.0)

        xhp = ctx.enter_context(tc.tile_pool(name="xhi", bufs=imgs + 1))
        xlp = ctx.enter_context(tc.tile_pool(name="xlo", bufs=imgs + 1))

        hi_t = {}
        lo_t = {}

        def load_x0():
            # img0 head/rest tiles.  The four head pieces are spread across
            # three DMA queues so their wire slots land first: hi-c0 leads
            # the Pool/SWDGE stream, hi-c1 + lo-c1 follow the sign pieces on
            # sync, lo-c0 is the scalar queue's only early DMA.  The rest
            # pieces ride the Pool stream behind the head.
            hi_h = xhp.tile([P, CC, HEAD], dt.float8e4, name="hi0h", tag="hih")
            lo_h = xlp.tile([P, CC, HEAD], dt.float8e4, name="lo0h", tag="loh")
            hi_r = xhp.tile([P, CC, FPAD - RST], dt.float8e4, name="hi0r", tag="hi")
            lo_r = xlp.tile([P, CC, FPAD - RST], dt.float8e4, name="lo0r", tag="lo")
            nc.gpsimd.dma_start(out=hi_h[:, 0, :], in_=xh_d[0, 0:P, 0:HEAD])
            nc.gpsimd.dma_start(out=lo_h[:, 0, :], in_=xl_d[0, 0:P, 0:HEAD])
            nc.sync.dma_start(out=hi_h[:, 1, :], in_=xh_d[0, P:2 * P, 0:HEAD])
            nc.scalar.dma_start(out=lo_h[:, 1, :], in_=xl_d[0, P:2 * P, 0:HEAD])
            for t, src in ((hi_r, xh_d), (lo_r, xl_d)):
                for cc in range(CC):
                    nc.gpsimd.dma_start(out=t[:, cc, :],
                                        in_=src[0, cc * P:(cc + 1) * P, RST:FPAD])
            hi_t[0] = (hi_h, hi_r, RST)
            lo_t[0] = (lo_h, lo_r, RST)

        def load_x(img):
            # Half-length pieces so store DMAs can interleave on the shared
            # wire between them.
            hi = xhp.tile([P, CC, FPAD], dt.float8e4, name=f"hi{img}", tag="hi")
            lo = xlp.tile([P, CC, FPAD], dt.float8e4, name=f"lo{img}", tag="lo")
            h2 = FPAD // 2
            for t, src in ((hi, xh_d), (lo, xl_d)):
                for a, b in ((0, h2), (h2, FPAD)):
                    for cc in range(CC):
                        nc.gpsimd.dma_start(out=t[:, cc, a:b],
                                            in_=src[img, cc * P:(cc + 1) * P, a:b])
            hi_t[img], lo_t[img] = (hi, hi, 0), (lo, lo, 0)

        def dma_w_signs(oo, cc, eng, thirds=False):
            # Sign-source (transposed, host-cast bf16 — sign-exact) quarters;
            # optionally in tap-thirds so the sign ops stream behind the
            # pieces.
            if thirds:
                for t3 in range(NT3):
                    eng.dma_start(
                        out=wts_sb[oo][cc][:, t3 * 3:t3 * 3 + 3]
                        .rearrange("p kk o -> p (kk o)"),
                        in_=wt_d[oo, cc, :, t3 * 3 * P:(t3 * 3 + 3) * P])
                return
            eng.dma_start(
                out=wts_sb[oo][cc].rearrange("p kk o -> p (kk o)"),
                in_=wt_d[oo, cc])

        def dma_w_scale(oo):
            h2 = CKK // 2
            nc.gpsimd.dma_start(out=w_sb[oo][0],
                                in_=w_d[oo * P:(oo + 1) * P, 0:h2])
            nc.gpsimd.dma_start(out=w_sb[oo][1],
                                in_=w_d[oo * P:(oo + 1) * P, h2:CKK])

        def sgn_w(oo, cc, t3):
            # wt8 = (w >= 0) - 0.5 in {-0.5, +0.5} (fp8-exact), one DVE op
            # per tap-third; the missing 2x folds into the eviction scale.
            nc.vector.tensor_scalar(
                out=wt8[oo][t3][:, cc], in0=wts_sb[oo][cc][:, t3 * 3:t3 * 3 + 3],
                scalar1=0.0, scalar2=0.5,
                op0=mybir.AluOpType.is_ge, op1=mybir.AluOpType.subtract)

        def reduce_scale(oo):
            # Per-out-channel scale column (DVE), in two halves so the first
            # can start as soon as its half of the source lands. 2/CKK
            # compensates the +-0.5 sign values.
            for h in range(2):
                nc.vector.tensor_reduce(
                    out=scale_p[oo][:, h:h + 1], in_=w_sb[oo][h],
                    axis=mybir.AxisListType.X,
                    op=mybir.AluOpType.add, apply_absolute_value=True)
            nc.vector.tensor_tensor(
                out=scale_sb[oo], in0=scale_p[oo][:, 0:1],
                in1=scale_p[oo][:, 1:2], op=mybir.AluOpType.add)
            nc.vector.tensor_scalar_mul(scale_sb[oo], scale_sb[oo], 2.0 / CKK)

        pp = ctx.enter_context(tc.tile_pool(name="psum", bufs=psum_bufs, space="PSUM"))
        op = ctx.enter_context(tc.tile_pool(name="ostage", bufs=10))

        def mm(ps, pair, oo, row0, nrows, k, n):
            ki, kj = divmod(k, KW)
            fs = (row0 + ki) * Wp + kj
            mv = nrows * Wp
            if row0 < 23:
                src_t = pair[0]
            else:
                src_t = pair[1]
                fs -= pair[2]
            if strided:
                rhs = (src_t[:, :, fs:fs + mv]
                       .rearrange("p c (h w) -> p c h w", w=Wp)[:, :, :, 0:W])
                out_ap = ps[:, 0:nrows * W]
            else:
                rhs = src_t[:, :, fs:fs + mv - 2]
                out_ap = ps[:, 0:mv - 2]
            nc.tensor.matmul(out_ap, lhsT=wt8[oo][k // 3][:, :, k % 3, :], rhs=rhs,
                             start=(n == 0), stop=(n == NMM - 1),
                             perf_mode=mybir.MatmulPerfMode.DoubleRow)

        def chunk_hi(img, oo, row0, nrows):
            pcols = nrows * W if strided else nrows * Wp
            ps = pp.tile([P, pcols], dt.float32, name=f"ps_{img}_{oo}_{row0}",
                         tag="ps")
            for n, k in enumerate(range(KK)):
                mm(ps, hi_t[img], oo, row0, nrows, k, n)
            return ps

        def chunk_lo_evict(img, oo, row0, nrows, store_eng, ps):
            n = KK
            for k in lo_taps:
                mm(ps, lo_t[img], oo, row0, nrows, k, n)
                n += 1
            st = op.tile([P, nrows, W], dt.float32,
                         name=f"st_{img}_{oo}_{row0}", tag=f"st{nrows}")
            psv = ps.rearrange("p (h w) -> p h w", w=W if strided else Wp)
            nc.scalar.mul(st, psv[:, :, 0:W], scale_sb[oo])
            store_eng.dma_start(
                out=y[img, oo * P:(oo + 1) * P, row0:row0 + nrows, :], in_=st)

        def conv_chunk(img, oo, row0, nrows, store_eng):
            ps = chunk_hi(img, oo, row0, nrows)
            chunk_lo_evict(img, oo, row0, nrows, store_eng, ps)

        def conv(img, mid=None, last=False, lead2=False):
            for oo in range(OO):
                final = last and oo == OO - 1
                chunks = [(i * hchunk, hchunk) for i in range(nch)]
                if final:
                    # split the very last chunk 6+2: the tiny tail's
                    # evict+store chain is what the kernel drain waits on.
                    chunks = chunks[:-1] + [(chunks[-1][0], 6),
                                            (chunks[-1][0] + 6, 2)]
                start_ci = 0
                if lead2 and oo == 0:
                    # A/B lead-in: both leading chunks' hi passes run before
                    # chunk0's lo taps, hiding the lo-head DMA latency.
                    ps0 = chunk_hi(img, 0, *chunks[0][:2])
                    ps1 = chunk_hi(img, 0, *chunks[1][:2])
                    chunk_lo_evict(img, 0, chunks[0][0], chunks[0][1],
                                   nc.sync, ps0)
                    chunk_lo_evict(img, 0, chunks[1][0], chunks[1][1],
                                   nc.sync, ps1)
                    start_ci = 2
                for ci, (row0, nrows) in enumerate(chunks):
                    if ci < start_ci:
                        continue
                    eng = nc.gpsimd if (final and ci == len(chunks) - 1) \
                        else nc.sync
                    conv_chunk(img, oo, row0, nrows, eng)
                    if mid is not None and oo == 1 and ci == 2:
                        mid()

        def late_weights_and_img1():
            # Deadline-ordered Pool/SWDGE stream behind img0's pieces:
            # oo1 sign sources (needed ~14us), oo0 scale halves (~15us),
            # img1's hi/lo (~22us), oo1 scale halves between.  DVE: oo1
            # signs first, then the slow scale reduces.
            hi = xhp.tile([P, CC, FPAD], dt.float8e4, name="hi1", tag="hi")
            lo = xlp.tile([P, CC, FPAD], dt.float8e4, name="lo1", tag="lo")
            dma_w_scale(0)
            dma_w_signs(1, 0, nc.gpsimd)
            dma_w_signs(1, 1, nc.gpsimd)
            for cc in range(CC):
                nc.gpsimd.dma_start(out=hi[:, cc, :],
                                    in_=xh_d[1, cc * P:(cc + 1) * P, :])
            dma_w_scale(1)
            for cc in range(CC):
                nc.gpsimd.dma_start(out=lo[:, cc, :],
                                    in_=xl_d[1, cc * P:(cc + 1) * P, :])
            hi_t[1], lo_t[1] = (hi, hi, 0), (lo, lo, 0)
            # DVE: interleave the scale-0 reduce halves with the oo1 sign
            # thirds so neither chain waits for the other to finish.
            h2 = CKK // 2
            nc.vector.tensor_reduce(
                out=scale_p[0][:, 0:1], in_=w_sb[0][0],
                axis=mybir.AxisListType.X,
                op=mybir.AluOpType.add, apply_absolute_value=True)
            sgn_w(1, 0, 0)
            sgn_w(1, 1, 0)
            nc.vector.tensor_reduce(
                out=scale_p[0][:, 1:2], in_=w_sb[0][1],
                axis=mybir.AxisListType.X,
                op=mybir.AluOpType.add, apply_absolute_value=True)
            sgn_w(1, 0, 1)
            sgn_w(1, 1, 1)
            nc.vector.tensor_tensor(
                out=scale_sb[0], in0=scale_p[0][:, 0:1],
                in1=scale_p[0][:, 1:2], op=mybir.AluOpType.add)
            nc.vector.tensor_scalar_mul(scale_sb[0], scale_sb[0], 2.0 / CKK)
            sgn_w(1, 0, 2)
            sgn_w(1, 1, 2)
            reduce_scale(1)

        # Emission order seeds per-engine program order: the oo0 sign-source
        # pieces lead the sync queue (they gate the first matmul), then
        # img0's pieces, then everything else on the Pool stream.
        dma_w_signs(0, 0, nc.sync)
        dma_w_signs(0, 1, nc.scalar)
        load_x0()
        late_weights_and_img1()
        # Dummy matmuls on the zeroed scratch keep the PE continuously busy
        # through its p-state ramp until the real stream is ready.
        with tc.high_priority():
            wps = pp.tile([P, 256], dt.float32, name="warm_ps", tag="ps")
            for _ in range(n_warm):
                nc.tensor.matmul(wps, lhsT=scratch[:, :, 0:P], rhs=scratch,
                                 start=True, stop=True,
                                 perf_mode=mybir.MatmulPerfMode.DoubleRow)
        with tc.high_priority():
            for t3 in range(NT3):
                sgn_w(0, 0, t3)
                sgn_w(0, 1, t3)

        conv(0, mid=(lambda: load_x(2)) if imgs > 2 else None, last=imgs == 1,
             lead2=True)
        for img in range(1, imgs):
            conv(img, mid=(lambda i=img: load_x(i + 2)) if img + 2 < imgs else None,
                 last=img == imgs - 1)
    nc.compile()
    return nc


BATCH, H, W = 32, 56, 56
N_CORES = 8
IMGS = BATCH // N_CORES
_NC_CACHE = {}


def _get_nc():
    key = (IMGS, H, W)
    if key not in _NC_CACHE:
        _NC_CACHE[key] = _build_conv_nc(IMGS, H, W, hchunk=8, psum_bufs=8)
    return _NC_CACHE[key]


def kernel(**inputs) -> np.ndarray:
    from concourse.bass_utils import run_bass_kernel_spmd
    import ml_dtypes

    x = np.asarray(inputs["x"], dtype=np.float32)
    weight = np.ascontiguousarray(np.asarray(inputs["weight"], dtype=np.float32))
    assert x.shape == (BATCH, IN_C, H, W), x.shape
    assert weight.shape == (OUT_C * CKK, 1), weight.shape

    # Host-side layout prep (data movement / dtype casts): halo-pad each image
    # channel to flat [58*58(+2)] lines and split into an e4m3 hi/lo pair
    # (hi = e4m3(x), lo = e4m3(x - hi), round-to-nearest); ship the weight
    # transposed bf16 (sign matrix in lhsT layout) plus a bf16 copy for the
    # scale reduction.
    Hp, Wp = H + 2, W + 2
    FPAD = Hp * Wp + 2
    xpad = np.zeros((BATCH, IN_C, FPAD), dtype=np.float32)
    xpad[:, :, :Hp * Wp].reshape(BATCH, IN_C, Hp, Wp)[:, :, 1:H + 1, 1:W + 1] = x
    xh = xpad.astype(ml_dtypes.float8_e4m3)
    xl = (xpad - xh.astype(np.float32)).astype(ml_dtypes.float8_e4m3)
    # Sign source as fp8: scale into e4m3's normal range and clip (sign-
    # exact monotone map; |w|*2^24 >= 30 for every weight, clip keeps all
    # values finite normals).
    wT = np.ascontiguousarray(
        np.clip(weight.reshape(OO, P, CC, P, KK).transpose(0, 2, 3, 4, 1)
                .reshape(OO, CC, P, KK * P) * 2.0**24, -224.0, 224.0)
        .astype(ml_dtypes.float8_e4m3))
    wB = np.ascontiguousarray(weight.reshape(OUT_C, CKK).astype(ml_dtypes.bfloat16))

    nc = _get_nc()
    in_maps = [
        {"xh": xh[c * IMGS:(c + 1) * IMGS], "xl": xl[c * IMGS:(c + 1) * IMGS],
         "w": wB, "wt": wT}
        for c in range(N_CORES)
    ]
    res = run_bass_kernel_spmd(nc, in_maps, core_ids=list(range(N_CORES)))
    return np.concatenate([res.results[c]["y"] for c in range(N_CORES)], axis=0)


# revision 50
# speedup vs baseline: 1.0003x; 1.0003x over previous
"""Binarized 3x3 conv (GeneralConv2d) on 8 NeuronCores — partial-residual fp8.

y[b,o,h,w] = mean_abs(w[o]) * sum_{c,kh,kw} sign(w[o,c,kh,kw]) * x[b,c,h+kh-1,w+kw-1]

Data-parallel over batch: 4 images per core on 8 cores; the tiny binarized
weight is replicated.  The conv runs on the tensor engine as fp8e4 DoubleRow
matmuls (256-deep contraction per instruction: the two in-channel chunks
ride in the k-tile pair dim), one halo-free 448-column window (8 rows x 56,
strided rhs AP) per instruction.

x ships from the host as an e4m3 hi/lo pair (hi = e4m3(x), lo = e4m3(x-hi),
both exact round-to-nearest) so no on-device residual math is needed.  The
error budget (rel 2e-2) does not require the full lo pass: only LO_TAPS of
the nine 3x3 taps get the lo-residual correction, which cuts the matmul
stream from 18 to 9+len(LO_TAPS) instructions per output chunk.  Error on
the fixed seeded inputs, measured on HW (bit-identical to the numpy model):
LO_TAPS=(0,2,4,8) -> rel 1.68e-2; (0,2,4,6,8) -> 1.66e-2; all nine -> 7e-4.

The sign matrix is computed on device as (w>=0)-0.5 = +-0.5 in a single DVE
op per tap-third (fp8-exact); the missing factor of 2 folds into the
per-channel eviction scale (2/CKK instead of 1/CKK).

Schedule notes (all verified against TimelineSim span traces):
- Dummy matmuls on a zeroed scratch keep the tensor engine continuously
  busy from t~1.5us until the real stream starts, so the p-state ramp
  (low/mid clock at the start of a busy run) is burnt on junk and every
  real matmul runs at full clock.
- DMA wire time is a single shared ~360GB/s resource and every DMA has
  ~2.8us of fixed issue+semaphore latency, so the startup-critical pieces
  (oo0 sign sources, img0 head) are spread over the sync/scalar/Pool
  queues in wire-deadline order, and everything else (oo1 signs, scale
  sources, img1) rides the Pool/SWDGE stream behind them, deadline-ordered.
- img0's x buffers are two overlapping head/rest tiles (chunks 0-2 read
  the head tile, rows>=23 the rest tile) so the first chunks gate only on
  the head DMAs; imgs 2-3 are prefetched at the middle of the previous
  image's oo1 pass.
- The first two chunks run hi-pass A/B style (both hi passes before the
  first lo taps) to hide the lo-head DMA latency.
- Evictions (PSUM scale-mul) run exclusively on ACT; stores ride the sync
  HWDGE queue.  The last two chunks are split into 4-row pieces (tail via
  the otherwise-idle Pool/SWDGE queue) so their smaller stores drain on
  the shared wire during the stream's tail, shortening the end-of-kernel
  drain chain.

Host-side layout prep (data movement / dtype casts only): x channels are
halo-padded to flat [58*58+2] lines so every load DMA is one contiguous
piece per partition; the weight ships transposed ([ckk, out]) as
clip(w*2^24, +-224) cast to e4m3 — a sign-exact monotone map that keeps
every weight a finite fp8 normal at half the bf16 wire cost — plus a bf16
copy for the on-device mean-abs scale reduction.
"""

import numpy as np

from contextlib import ExitStack

import concourse.mybir as mybir
from concourse import bacc
import concourse.tile as tile

dt = mybir.dt
OUT_C = 256
IN_C = 256
KH = KW = 3
KK = KH * KW           # 9
CKK = IN_C * KK        # 2304
P = 128
CC = IN_C // P         # 2 in-channel chunks (the DoubleRow k-tile pair)
OO = OUT_C // P        # 2 out-channel chunks

# Taps (kh*3+kw) that get the lo-residual correction pass.
LO_TAPS = (0, 2, 4, 8)
STRIDED = True         # 448-col halo-free matmuls (4-dim rhs AP)
N_WARM = 30            # dummy matmuls covering the p-state ramp window


def _build_conv_nc(imgs: int, H: int, W: int, hchunk: int = 8, psum_bufs: int = 8,
                   lo_taps=LO_TAPS, strided=STRIDED, n_warm=N_WARM):
    assert H % hchunk == 0
    nch = H // hchunk
    Hp, Wp = H + 2, W + 2
    FLAT = Hp * Wp         # 3364
    FPAD = FLAT + 2        # +2 so the last tap window stays in-bounds
    NMM = KK + len(lo_taps)
    NT3 = KK // 3          # tap-thirds
    nc = bacc.Bacc("TRN2", target_bir_lowering=False, debug=False,
                   enable_asserts=False, num_devices=8)
    xh_d = nc.declare_dram_parameter("xh", [imgs, IN_C, FPAD], dt.float8e4,
                                     isOutput=False)
    xl_d = nc.declare_dram_parameter("xl", [imgs, IN_C, FPAD], dt.float8e4,
                                     isOutput=False)
    w_d = nc.declare_dram_parameter("w", [OUT_C, CKK], dt.bfloat16, isOutput=False)
    wt_d = nc.declare_dram_parameter("wt", [OO, CC, P, KK * P], dt.float8e4,
                                     isOutput=False)
    y = nc.declare_dram_parameter("y", [imgs, OUT_C, H, W], dt.float32, isOutput=True)

    # img0 head/rest split: head covers chunks 0..2 (windows end <= 1508),
    # rest covers rows >= 23 (chunks 3..6 read windows in [1392, 3364]).
    HEAD = 1510
    RST = 1334             # rest tile holds flat cols [RST:FPAD]

    with tile.TileContext(nc) as tc, ExitStack() as ctx:
        wprep = ctx.enter_context(tc.tile_pool(name="wprep", bufs=1))
        w_sb = [[wprep.tile([P, CKK // 2], dt.bfloat16, name=f"w_sb{o}_{h}")
                 for h in range(2)] for o in range(OO)]
        scale_p = [wprep.tile([P, 2], dt.float32, name=f"scale_p{o}")
                   for o in range(OO)]
        wts_sb = [[wprep.tile([P, KK, P], dt.float8e4, name=f"wts{o}_{c}")
                   for c in range(CC)] for o in range(OO)]
        wt8 = [[wprep.tile([P, CC, 3, P], dt.float8e4, name=f"wt8_{o}_{t}")
                for t in range(NT3)] for o in range(OO)]
        scale_sb = [wprep.tile([P, 1], dt.float32, name=f"scale{o}")
                    for o in range(OO)]
        scratch = wprep.tile([P, CC, 256], dt.float8e4, name="scratch")
        nc.vector.memset(scratch, 0.0)

        xhp = ctx.enter_context(tc.tile_pool(name="xhi", bufs=imgs + 1))
        xlp = ctx.enter_context(tc.tile_pool(name="xlo", bufs=imgs + 1))

        hi_t = {}
        lo_t = {}

        def load_x0():
            # img0 head/rest tiles.  The four head pieces are spread across
            # three DMA queues so their wire slots land first: hi-c0 leads
            # the Pool/SWDGE stream, hi-c1 + lo-c1 follow the sign pieces on
            # sync, lo-c0 is the scalar queue's only early DMA.  The rest
            # pieces ride the Pool stream behind the head.
            hi_h = xhp.tile([P, CC, HEAD], dt.float8e4, name="hi0h", tag="hih")
            lo_h = xlp.tile([P, CC, HEAD], dt.float8e4, name="lo0h", tag="loh")
            hi_r = xhp.tile([P, CC, FPAD - RST], dt.float8e4, name="hi0r", tag="hi")
            lo_r = xlp.tile([P, CC, FPAD - RST], dt.float8e4, name="lo0r", tag="lo")
            nc.gpsimd.dma_start(out=hi_h[:, 0, :], in_=xh_d[0, 0:P, 0:HEAD])
            nc.gpsimd.dma_start(out=lo_h[:, 0, :], in_=xl_d[0, 0:P, 0:HEAD])
            nc.sync.dma_start(out=hi_h[:, 1, :], in_=xh_d[0, P:2 * P, 0:HEAD])
            nc.scalar.dma_start(out=lo_h[:, 1, :], in_=xl_d[0, P:2 * P, 0:HEAD])
            for t, src in ((hi_r, xh_d), (lo_r, xl_d)):
                for cc in range(CC):
                    nc.gpsimd.dma_start(out=t[:, cc, :],
                                        in_=src[0, cc * P:(cc + 1) * P, RST:FPAD])
            hi_t[0] = (hi_h, hi_r, RST)
            lo_t[0] = (lo_h, lo_r, RST)

        def load_x(img):
            # Half-length pieces so store DMAs can interleave on the shared
            # wire between them.
            hi = xhp.tile([P, CC, FPAD], dt.float8e4, name=f"hi{img}", tag="hi")
            lo = xlp.tile([P, CC, FPAD], dt.float8e4, name=f"lo{img}", tag="lo")
            h2 = FPAD // 2
            for t, src in ((hi, xh_d), (lo, xl_d)):
                for a, b in ((0, h2), (h2, FPAD)):
                    for cc in range(CC):
                        nc.gpsimd.dma_start(out=t[:, cc, a:b],
                                            in_=src[img, cc * P:(cc + 1) * P, a:b])
            hi_t[img], lo_t[img] = (hi, hi, 0), (lo, lo, 0)

        def dma_w_signs(oo, cc, eng, thirds=False):
            # Sign-source (transposed, host-cast bf16 — sign-exact) quarters;
            # optionally in tap-thirds so the sign ops stream behind the
            # pieces.
            if thirds:
                for t3 in range(NT3):
                    eng.dma_start(
                        out=wts_sb[oo][cc][:, t3 * 3:t3 * 3 + 3]
                        .rearrange("p kk o -> p (kk o)"),
                        in_=wt_d[oo, cc, :, t3 * 3 * P:(t3 * 3 + 3) * P])
                return
            eng.dma_start(
                out=wts_sb[oo][cc].rearrange("p kk o -> p (kk o)"),
                in_=wt_d[oo, cc])

        def dma_w_scale(oo):
            h2 = CKK // 2
            nc.gpsimd.dma_start(out=w_sb[oo][0],
                                in_=w_d[oo * P:(oo + 1) * P, 0:h2])
            nc.gpsimd.dma_start(out=w_sb[oo][1],
                                in_=w_d[oo * P:(oo + 1) * P, h2:CKK])

        def sgn_w(oo, cc, t3):
            # wt8 = (w >= 0) - 0.5 in {-0.5, +0.5} (fp8-exact), one DVE op
            # per tap-third; the missing 2x folds into the eviction scale.
            nc.vector.tensor_scalar(
                out=wt8[oo][t3][:, cc], in0=wts_sb[oo][cc][:, t3 * 3:t3 * 3 + 3],
                scalar1=0.0, scalar2=0.5,
                op0=mybir.AluOpType.is_ge, op1=mybir.AluOpType.subtract)

        def reduce_scale(oo):
            # Per-out-channel scale column (DVE), in two halves so the first
            # can start as soon as its half of the source lands. 2/CKK
            # compensates the +-0.5 sign values.
            for h in range(2):
                nc.vector.tensor_reduce(
                    out=scale_p[oo][:, h:h + 1], in_=w_sb[oo][h],
                    axis=mybir.AxisListType.X,
                    op=mybir.AluOpType.add, apply_absolute_value=True)
            nc.vector.tensor_tensor(
                out=scale_sb[oo], in0=scale_p[oo][:, 0:1],
                in1=scale_p[oo][:, 1:2], op=mybir.AluOpType.add)
            nc.vector.tensor_scalar_mul(scale_sb[oo], scale_sb[oo], 2.0 / CKK)

        pp = ctx.enter_context(tc.tile_pool(name="psum", bufs=psum_bufs, space="PSUM"))
        op = ctx.enter_context(tc.tile_pool(name="ostage", bufs=10))

        def mm(ps, pair, oo, row0, nrows, k, n):
            ki, kj = divmod(k, KW)
            fs = (row0 + ki) * Wp + kj
            mv = nrows * Wp
            if row0 < 23:
                src_t = pair[0]
            else:
                src_t = pair[1]
                fs -= pair[2]
            if strided:
                rhs = (src_t[:, :, fs:fs + mv]
                       .rearrange("p c (h w) -> p c h w", w=Wp)[:, :, :, 0:W])
                out_ap = ps[:, 0:nrows * W]
            else:
                rhs = src_t[:, :, fs:fs + mv - 2]
                out_ap = ps[:, 0:mv - 2]
            nc.tensor.matmul(out_ap, lhsT=wt8[oo][k // 3][:, :, k % 3, :], rhs=rhs,
                             start=(n == 0), stop=(n == NMM - 1),
                             perf_mode=mybir.MatmulPerfMode.DoubleRow)

        def chunk_hi(img, oo, row0, nrows):
            pcols = nrows * W if strided else nrows * Wp
            ps = pp.tile([P, pcols], dt.float32, name=f"ps_{img}_{oo}_{row0}",
                         tag="ps")
            for n, k in enumerate(range(KK)):
                mm(ps, hi_t[img], oo, row0, nrows, k, n)
            return ps

        def chunk_lo_evict(img, oo, row0, nrows, store_eng, ps):
            n = KK
            for k in lo_taps:
                mm(ps, lo_t[img], oo, row0, nrows, k, n)
                n += 1
            st = op.tile([P, nrows, W], dt.float32,
                         name=f"st_{img}_{oo}_{row0}", tag=f"st{nrows}")
            psv = ps.rearrange("p (h w) -> p h w", w=W if strided else Wp)
            nc.scalar.mul(st, psv[:, :, 0:W], scale_sb[oo])
            store_eng.dma_start(
                out=y[img, oo * P:(oo + 1) * P, row0:row0 + nrows, :], in_=st)

        def conv_chunk(img, oo, row0, nrows, store_eng):
            ps = chunk_hi(img, oo, row0, nrows)
            chunk_lo_evict(img, oo, row0, nrows, store_eng, ps)

        def conv(img, mid=None, last=False, lead2=False):
            for oo in range(OO):
                final = last and oo == OO - 1
                chunks = [(i * hchunk, hchunk) for i in range(nch)]
                if final:
                    # split the last two chunks into 4-row pieces: their
                    # smaller stores drain on the shared wire during the
                    # stream's tail instead of after the last matmul.
                    chunks = chunks[:-2] + [(40, 4), (44, 4), (48, 4),
                                            (52, 4)]
                start_ci = 0
                if lead2 and oo == 0:
                    # A/B lead-in: both leading chunks' hi passes run before
                    # chunk0's lo taps, hiding the lo-head DMA latency.
                    ps0 = chunk_hi(img, 0, *chunks[0][:2])
                    ps1 = chunk_hi(img, 0, *chunks[1][:2])
                    chunk_lo_evict(img, 0, chunks[0][0], chunks[0][1],
                                   nc.sync, ps0)
                    chunk_lo_evict(img, 0, chunks[1][0], chunks[1][1],
                                   nc.sync, ps1)
                    start_ci = 2
                for ci, (row0, nrows) in enumerate(chunks):
                    if ci < start_ci:
                        continue
                    eng = nc.gpsimd if (final and ci == len(chunks) - 1) \
                        else nc.sync
                    conv_chunk(img, oo, row0, nrows, eng)
                    if mid is not None and oo == 1 and ci == 2:
                        mid()

        def late_weights_and_img1():
            # Deadline-ordered Pool/SWDGE stream behind img0's pieces:
            # oo1 sign sources (needed ~14us), oo0 scale halves (~15us),
            # img1's hi/lo (~22us), oo1 scale halves between.  DVE: oo1
            # signs first, then the slow scale reduces.
            hi = xhp.tile([P, CC, FPAD], dt.float8e4, name="hi1", tag="hi")
            lo = xlp.tile([P, CC, FPAD], dt.float8e4, name="lo1", tag="lo")
            dma_w_scale(0)
            dma_w_signs(1, 0, nc.gpsimd)
            dma_w_signs(1, 1, nc.gpsimd)
            for cc in range(CC):
                nc.gpsimd.dma_start(out=hi[:, cc, :],
                                    in_=xh_d[1, cc * P:(cc + 1) * P, :])
            dma_w_scale(1)
            for cc in range(CC):
                nc.gpsimd.dma_start(out=lo[:, cc, :],
                                    in_=xl_d[1, cc * P:(cc + 1) * P, :])
            hi_t[1], lo_t[1] = (hi, hi, 0), (lo, lo, 0)
            # DVE: interleave the scale-0 reduce halves with the oo1 sign
            # thirds so neither chain waits for the other to finish.
            h2 = CKK // 2
            nc.vector.tensor_reduce(
                out=scale_p[0][:, 0:1], in_=w_sb[0][0],
                axis=mybir.AxisListType.X,
                op=mybir.AluOpType.add, apply_absolute_value=True)
            sgn_w(1, 0, 0)
            sgn_w(1, 1, 0)
            nc.vector.tensor_reduce(
                out=scale_p[0][:, 1:2], in_=w_sb[0][1],
                axis=mybir.AxisListType.X,
                op=mybir.AluOpType.add, apply_absolute_value=True)
            sgn_w(1, 0, 1)
            sgn_w(1, 1, 1)
            nc.vector.tensor_tensor(
                out=scale_sb[0], in0=scale_p[0][:, 0:1],
                in1=scale_p[0][:, 1:2], op=mybir.AluOpType.add)
            nc.vector.tensor_scalar_mul(scale_sb[0], scale_sb[0], 2.0 / CKK)
            sgn_w(1, 0, 2)
            sgn_w(1, 1, 2)
            reduce_scale(1)

        # Emission order seeds per-engine program order: the oo0 sign-source
        # pieces lead the sync queue (they gate the first matmul), then
        # img0's pieces, then everything else on the Pool stream.
        dma_w_signs(0, 0, nc.sync)
        dma_w_signs(0, 1, nc.scalar)
        load_x0()
        late_weights_and_img1()
        # Dummy matmuls on the zeroed scratch keep the PE continuously busy
        # through its p-state ramp until the real stream is ready.
        with tc.high_priority():
            wps = pp.tile([P, 256], dt.float32, name="warm_ps", tag="ps")
            for _ in range(n_warm):
                nc.tensor.matmul(wps, lhsT=scratch[:, :, 0:P], rhs=scratch,
                                 start=True, stop=True,
                                 perf_mode=mybir.MatmulPerfMode.DoubleRow)
        with tc.high_priority():
            for t3 in range(NT3):
                sgn_w(0, 0, t3)
                sgn_w(0, 1, t3)

        conv(0, mid=(lambda: load_x(2)) if imgs > 2 else None, last=imgs == 1,
             lead2=True)
        for img in range(1, imgs):
            conv(img, mid=(lambda i=img: load_x(i + 2)) if img + 2 < imgs else None,
                 last=img == imgs - 1)
    nc.compile()
    return nc


BATCH, H, W = 32, 56, 56
N_CORES = 8
IMGS = BATCH // N_CORES
_NC_CACHE = {}


def _get_nc():
    key = (IMGS, H, W)
    if key not in _NC_CACHE:
        _NC_CACHE[key] = _build_conv_nc(IMGS, H, W, hchunk=8, psum_bufs=8)
    return _NC_CACHE[key]


def kernel(**inputs) -> np.ndarray:
    from concourse.bass_utils import run_bass_kernel_spmd
    import ml_dtypes

    x = np.asarray(inputs["x"], dtype=np.float32)
    weight = np.ascontiguousarray(np.asarray(inputs["weight"], dtype=np.float32))
    assert x.shape == (BATCH, IN_C, H, W), x.shape
    assert weight.shape == (OUT_C * CKK, 1), weight.shape

    # Host-side layout prep (data movement / dtype casts): halo-pad each image
    # channel to flat [58*58(+2)] lines and split into an e4m3 hi/lo pair
    # (hi = e4m3(x), lo = e4m3(x - hi), round-to-nearest); ship the weight
    # transposed bf16 (sign matrix in lhsT layout) plus a bf16 copy for the
    # scale reduction.
    Hp, Wp = H + 2, W + 2
    FPAD = Hp * Wp + 2
    xpad = np.zeros((BATCH, IN_C, FPAD), dtype=np.float32)
    xpad[:, :, :Hp * Wp].reshape(BATCH, IN_C, Hp, Wp)[:, :, 1:H + 1, 1:W + 1] = x
    xh = xpad.astype(ml_dtypes.float8_e4m3)
    xl = (xpad - xh.astype(np.float32)).astype(ml_dtypes.float8_e4m3)
    # Sign source as fp8: scale into e4m3's normal range and clip (sign-
    # exact monotone map; |w|*2^24 >= 30 for every weight, clip keeps all
    # values finite normals).
    wT = np.ascontiguousarray(
        np.clip(weight.reshape(OO, P, CC, P, KK).transpose(0, 2, 3, 4, 1)
                .reshape(OO, CC, P, KK * P) * 2.0**24, -224.0, 224.0)
        .astype(ml_dtypes.float8_e4m3))
    wB = np.ascontiguousarray(weight.reshape(OUT_C, CKK).astype(ml_dtypes.bfloat16))

    nc = _get_nc()
    in_maps = [
        {"xh": xh[c * IMGS:(c + 1) * IMGS], "xl": xl[c * IMGS:(c + 1) * IMGS],
         "w": wB, "wt": wT}
        for c in range(N_CORES)
    ]
    res = run_bass_kernel_spmd(nc, in_maps, core_ids=list(range(N_CORES)))
    return np.concatenate([res.results[c]["y"] for c in range(N_CORES)], axis=0)
